# revision 17
# baseline (speedup 1.0000x reference)
"""Trainium2 Bass kernel for nn_CausalSelfAttention (GQA + RoPE + qk-RMSNorm).

Strategy (Megatron-style head parallelism over 8 NeuronCores):
  - Each core owns 2 of the 16 q heads and the matching 1 of 8 kv heads.
  - Per core: QKV projection for its 512 rows of w_attn, RoPE + qk RMS norm,
    causal flash-style attention for its (2 q heads x 2 batches), and a
    partial output projection through its 256 columns of w_proj.
  - Host sums the 8 partial outputs (no on-device collectives).

v2 kernel structure (vs v1):
  - QKV loops k-innermost with n-pairs so each stationary w-tile streams
    1024 columns (LDWEIGHTS fully hidden), PSUM double-buffered.
  - Attention interleaves the two q heads (units A/B): independent
    score->exp->AV chains hide the Act exp latency, and A/B share the
    k/vT/ones stationaries so LDWEIGHTS are amortized.
  - Causal diagonal j-tiles only compute/read the valid column subranges
    in the AV and denominator matmuls (no memsets needed).
  - exp runs on whole [2, 512] pair tiles (fewer Act instructions).
  - Output projection reuses each yT stationary across 4 output groups.

All tensors are fed to the device pre-swizzled into SBUF-ready
[128, free...] layouts (bf16 for matmul operands).  Matmuls run in bf16 with
fp32 PSUM accumulation; softmax/statistics run in fp32.

Self-contained: hardcodes all shapes from the problem spec.
"""

import math
import numpy as np
import ml_dtypes
from contextlib import ExitStack

# ---- problem constants (hardcoded per spec) ----
B, T, C = 2, 2048, 2048
N_HEAD, N_KV_HEAD, HD = 16, 8, 128
KV_DIM = N_KV_HEAD * HD
EPS = 1.1920929e-07
N_CORES = 8
QH_PER_CORE = N_HEAD // N_CORES          # 2
TOK = B * T                              # 4096
P = 128
TG = 512                                 # token group (matmul N)
NT = TOK // TG                           # 8 token groups
KT = C // P                              # 16 contraction tiles
NGB = T // TG                            # 4 q groups per batch
NJB = T // P                             # 16 k tiles per batch
SCALE = 1.0 / math.sqrt(HD)

BF16 = ml_dtypes.bfloat16

_CACHE = {}


# --------------------------------------------------------------------------
# device program
# --------------------------------------------------------------------------

def _emit(tc, out_ap, t_in):
    import concourse.bass as bass  # noqa: F401
    import concourse.mybir as mybir

    f32 = mybir.dt.float32
    bf16 = mybir.dt.bfloat16
    AF = mybir.ActivationFunctionType
    nc = tc.nc

    x_d = t_in["x_sw"]
    wq_d = t_in["wq_sw"]
    wp_d = t_in["wp_sw"]
    cs_d = t_in["cs_sw"]
    mask_d = t_in["mask_sw"]
    eye_d = t_in["eye_sw"]

    NQUAD = 4            # 1024-token quads
    NPQ = 2              # n-groups (512 tok) per quad

    with ExitStack() as root:
        const = root.enter_context(tc.tile_pool(name="const", bufs=1))
        xin = root.enter_context(tc.tile_pool(name="xin", bufs=2))

        # ---- DMA ladder: k-ascending chunks of x (sync queue) and wq
        # (scalar queue) so the first QKV k-loop streams without stalls.
        # Only the two hardware DGE queues carry bulk data (gpsimd DMA is
        # software-DGE and slow).
        wq_sb = const.tile([P, KT, TG], bf16)
        xq_sb = [None] * NQUAD
        xq_sb[0] = xin.tile([P, KT, NPQ, TG], bf16, name="xq")
        # sync queue feeds x k0-7 in fine chunks; scalar queue carries wq
        # then x k8-15, so both ladders stay ahead of the k-loop. Consts
        # follow on sync (finished before rope/attention need them).
        nc.scalar.dma_start(out=wq_sb[:, 0:1, :], in_=wq_d[:, 0:1, :])
        nc.sync.dma_start(out=xq_sb[0][:, 0:1, :, :], in_=x_d[:, 0:1, 0:2, :])
        nc.scalar.dma_start(out=wq_sb[:, 1:4, :], in_=wq_d[:, 1:4, :])
        nc.sync.dma_start(out=xq_sb[0][:, 1:3, :, :], in_=x_d[:, 1:3, 0:2, :])
        nc.scalar.dma_start(out=wq_sb[:, 4:16, :], in_=wq_d[:, 4:16, :])
        nc.sync.dma_start(out=xq_sb[0][:, 3:5, :, :], in_=x_d[:, 3:5, 0:2, :])
        nc.sync.dma_start(out=xq_sb[0][:, 5:8, :, :], in_=x_d[:, 5:8, 0:2, :])
        nc.scalar.dma_start(out=xq_sb[0][:, 8:12, :, :], in_=x_d[:, 8:12, 0:2, :])
        nc.scalar.dma_start(out=xq_sb[0][:, 12:16, :, :],
                            in_=x_d[:, 12:16, 0:2, :])

        eye_sb = const.tile([P, P], bf16)
        nc.sync.dma_start(out=eye_sb[:], in_=eye_d)
        cs_sb = const.tile([P, 2, T], bf16)
        nc.sync.dma_start(out=cs_sb[:], in_=cs_d)
        mask_sb = const.tile([P, P], bf16)
        nc.sync.dma_start(out=mask_sb[:], in_=mask_d)
        wp_sb = const.tile([P, QH_PER_CORE, C], bf16)
        nc.sync.dma_start(out=wp_sb[:], in_=wp_d)
        eps_sb = const.tile([P, 1], f32)
        nc.vector.memset(eps_sb[:], EPS)
        onesm_sb = const.tile([P, P], bf16)
        nc.vector.memset(onesm_sb[:], 1.0)

        big = root.enter_context(tc.tile_pool(name="big", bufs=1))
        ropet = root.enter_context(tc.tile_pool(name="ropet", bufs=1))

        def rope_m(b, m):
            sl = slice(b * T, (b + 1) * T)
            t1 = ropet.tile([P, T], bf16, tag="t1", name="t1")
            xsw = ropet.tile([P, T], bf16, tag="xsw", name="xsw")
            nc.gpsimd.dma_start(out=xsw[0:64, :], in_=qn[m][64:128, sl])
            nc.gpsimd.dma_start(out=xsw[64:128, :], in_=qn[m][0:64, sl])
            nc.vector.tensor_mul(t1[:], qn[m][:, sl], cs_sb[:, 0])
            # t1 = [x1*c ; x2*c]; xsw*s2n = [x2*s ; -x1*s]
            nc.vector.tensor_mul(xsw[:], xsw[:], cs_sb[:, 1])
            nc.vector.tensor_add(qn[m][:, sl], t1[:], xsw[:])

        def rope_batch(b):
            # k first: attention needs it earliest
            for m in (2, 0, 1):
                rope_m(b, m)
        # post-rope, post-norm q (2 heads) and k, in [d, tok] layout
        qn = [big.tile([P, TOK], bf16, name=f"qn{m}", tag=f"qn{m}") for m in range(3)]
        v_sb = big.tile([P, TOK], bf16, tag="v")
        vT_sb = big.tile([P, 2 * NJB, P], bf16, tag="vT")   # [ktok, (b,j), d]
        yT = [big.tile([P, TOK], bf16, name=f"yT{h}", tag=f"yT{h}") for h in range(QH_PER_CORE)]

        # ------- stage 1: QKV projection + rms-norm + rope + v transpose ----
        with ExitStack() as s1:
            qkv_ps = s1.enter_context(tc.tile_pool(name="qkvps", bufs=4, space="PSUM"))
            ssq_ps = s1.enter_context(tc.tile_pool(name="ssqps", bufs=2, space="PSUM"))
            vt_ps = s1.enter_context(tc.tile_pool(name="vtps", bufs=2, space="PSUM"))
            sqp = s1.enter_context(tc.tile_pool(name="sq", bufs=3))
            qrp = s1.enter_context(tc.tile_pool(name="qr", bufs=3))
            srp = s1.enter_context(tc.tile_pool(name="sr", bufs=3))

            def vt_batch(b):
                for blk in range(b * NJB, (b + 1) * NJB):
                    tp = vt_ps.tile([P, P], bf16, name='tp')
                    nc.tensor.transpose(
                        tp[:], v_sb[:, blk * P:(blk + 1) * P], eye_sb[:]
                    )
                    nc.vector.tensor_copy(vT_sb[:, blk], tp[:])

            for q in range(NQUAD):
                if xq_sb[q] is None:
                    xq_sb[q] = xin.tile([P, KT, NPQ, TG], bf16, name="xq")
                    nc.sync.dma_start(
                        out=xq_sb[q][:],
                        in_=x_d[:, :, NPQ * q:NPQ * (q + 1), :],
                    )
                # prefetch next quad
                nq = q + 1
                if nq < NQUAD and xq_sb[nq] is None:
                    xq_sb[nq] = xin.tile([P, KT, NPQ, TG], bf16, name="xq")
                    nc.sync.dma_start(
                        out=xq_sb[nq][:],
                        in_=x_d[:, :, NPQ * nq:NPQ * (nq + 1), :],
                    )
                if q == 0:
                    # first quad: run m0+m1 together in two k-half passes so
                    # the PE only needs the first half of x/wq early (the
                    # initial DMA ladder is still streaming the rest in)
                    psq0 = {
                        m: [qkv_ps.tile([P, TG], f32, name=f'ps{m}{_n}', tag='ps')
                            for _n in range(NPQ)]
                        for m in (0, 1)
                    }
                    for kh in (0, 1):
                        for k in range(8 * kh, 8 * kh + 8):
                            for m in (0, 1):
                                for n in range(NPQ):
                                    nc.tensor.matmul(
                                        psq0[m][n][:],
                                        wq_sb[:, k, m * P:(m + 1) * P],
                                        xq_sb[q][:, k, n, :],
                                        start=(k == 0),
                                        stop=(k == KT - 1),
                                    )
                for m in range(4):
                    if q == 0 and m in (0, 1):
                        ps = psq0[m]
                    else:
                        ps = [qkv_ps.tile([P, TG], f32, name=f'ps{_n}', tag='ps') for _n in range(NPQ)]
                        for k in range(KT):
                            for n in range(NPQ):
                                nc.tensor.matmul(
                                    ps[n][:],
                                    wq_sb[:, k, m * P:(m + 1) * P],
                                    xq_sb[q][:, k, n, :],
                                    start=(k == 0),
                                    stop=(k == KT - 1),
                                )
                    for n in range(NPQ):
                        ts = q * NPQ * TG + n * TG
                        if m == 3:
                            nc.scalar.copy(v_sb[:, ts:ts + TG], ps[n][:])
                        else:
                            # rms-norm: broadcast sum-of-squares via ones MM.
                            # Both psum reads happen on Act so the bank frees
                            # independently of DVE backlog (rope bursts).
                            sq = sqp.tile([P, TG], bf16, name='sq')
                            nc.scalar.activation(sq[:], ps[n][:], AF.Square)
                            qr = qrp.tile([P, TG], bf16, name='qr')
                            nc.scalar.copy(qr[:], ps[n][:])
                            ssqb = ssq_ps.tile([P, TG], f32)
                            nc.tensor.matmul(
                                ssqb[:], onesm_sb[:], sq[:], start=True, stop=True
                            )
                            srb = srp.tile([P, TG], f32, name='srb')
                            nc.scalar.activation(
                                srb[:], ssqb[:], AF.Sqrt,
                                bias=eps_sb[:], scale=1.0 / HD,
                            )
                            nc.vector.reciprocal_approx_fast(srb[:], srb[:])
                            # normalized copy -> sbuf (rope comes after;
                            # rotation commutes with the per-token scale)
                            nc.vector.tensor_mul(
                                qn[m][:, ts:ts + TG], qr[:], srb[:]
                            )
                if q == 1:
                    rope_batch(0)
                    vt_batch(0)
                elif q == 3:
                    # rope(b1) is deferred into the attention(b0) stream;
                    # only the k head is roped here (cheap, needed first).
                    # preload the exp activation table while PE transposes v.
                    scr = sqp.tile([P, 1], f32, name='scr')
                    nc.scalar.activation(scr[:], eps_sb[:], AF.Exp)
                    vt_batch(1)
                    rope_m(1, 2)

        # ---------------- stage 2: attention ------------------------------
        # batch 0 runs both q heads interleaved (2-unit). batch 1 runs the
        # heads serially (1-unit) with batch-0 output-projection tiles
        # interleaved as PE filler; their out-DMAs overlap attention.
        with ExitStack() as s3:
            s_ps = s3.enter_context(tc.tile_pool(name="sps", bufs=2, space="PSUM"))
            ptp = s3.enter_context(tc.tile_pool(name="pt", bufs=16))
            denp = s3.enter_context(tc.tile_pool(name="den", bufs=2))

            def attn_group(b, g, units, y_pool, d_pool, ilv_den=True):
                qsl = slice(b * T + g * TG, b * T + (g + 1) * TG)
                jmax = 4 * g + 3
                npr = (jmax + 1) // 2
                pts = {u: [] for u in units}
                for pr in range(npr):
                    offp = (2 * pr - 4 * g) * P if 2 * pr >= 4 * g else 0
                    sp = {u: s_ps.tile([P, 2, TG], f32, name=f'sp{u}', tag='sp')
                          for u in units}
                    for jj in (0, 1):
                        j = 2 * pr + jj
                        kblk = qn[2][:, b * T + j * P: b * T + (j + 1) * P]
                        for u in units:
                            nc.tensor.matmul(
                                sp[u][:, jj, offp:],
                                kblk,
                                qn[u][:, qsl][:, offp:],
                                start=True,
                                stop=True,
                            )
                    for u in units:
                        pt = ptp.tile([P, 2, TG], bf16, name='pt')
                        nc.scalar.activation(
                            pt[:, :, offp:], sp[u][:, :, offp:],
                            AF.Exp, scale=SCALE,
                        )
                        for jj in (0, 1):
                            j = 2 * pr + jj
                            if j >= 4 * g:
                                offj = (j - 4 * g) * P
                                nc.vector.tensor_mul(
                                    pt[:, jj, offj:offj + P],
                                    pt[:, jj, offj:offj + P],
                                    mask_sb[:],
                                )
                            pts[u].append((jj, pt))
                yp = {u: y_pool.tile([P, TG], f32, name=f'yp{u}', tag='yp')
                      for u in units}
                dp = {u: d_pool.tile([P, TG], f32, name=f'dp{u}', tag='dp')
                      for u in units}
                # den interleaved with AV per j (2-unit): both finish with
                # the group, so normalize overlaps the next group's scores.
                # For 1-unit groups the interleave would alternate the
                # stationary every single matmul (LDWEIGHTS thrash), so den
                # runs as a separate ones-stationary pass instead.
                for j in range(jmax + 1):
                    offj = (j - 4 * g) * P if j >= 4 * g else 0
                    vblk = vT_sb[:, b * NJB + j]
                    for u in units:
                        jj, pt = pts[u][j]
                        nc.tensor.matmul(
                            yp[u][:, offj:], vblk, pt[:, jj, offj:],
                            start=(j == 0), stop=(j == jmax),
                        )
                    if ilv_den:
                        for u in units:
                            jj, pt = pts[u][j]
                            nc.tensor.matmul(
                                dp[u][:, offj:], onesm_sb[:], pt[:, jj, offj:],
                                start=(j == 0), stop=(j == jmax),
                            )
                if not ilv_den:
                    for u in units:
                        for j in range(jmax + 1):
                            offj = (j - 4 * g) * P if j >= 4 * g else 0
                            jj, pt = pts[u][j]
                            nc.tensor.matmul(
                                dp[u][:, offj:], onesm_sb[:], pt[:, jj, offj:],
                                start=(j == 0), stop=(j == jmax),
                            )
                for u in units:
                    den = denp.tile([P, TG], f32, name='den')
                    nc.vector.reciprocal_approx_fast(den[:], dp[u][:])
                    nc.vector.tensor_mul(yT[u][:, qsl], yp[u][:], den[:])

            with ExitStack() as s3a:
                y_ps = s3a.enter_context(
                    tc.tile_pool(name="yps", bufs=2, space="PSUM"))
                d_ps = s3a.enter_context(
                    tc.tile_pool(name="dps", bufs=2, space="PSUM"))
                for g in range(NGB):
                    if g in (1, 2):
                        rope_m(1, g - 1)   # deferred rope of batch-1 q heads
                    attn_group(0, g, (0, 1), y_ps, d_ps)

            with ExitStack() as s3b:
                y1_ps = s3b.enter_context(
                    tc.tile_pool(name="y1ps", bufs=1, space="PSUM"))
                d1_ps = s3b.enter_context(
                    tc.tile_pool(name="d1ps", bufs=1, space="PSUM"))
                o2_ps = s3b.enter_context(
                    tc.tile_pool(name="o2ps", bufs=2, space="PSUM"))
                ost2p = s3b.enter_context(tc.tile_pool(name="ost2", bufs=2))

                def _proj_ogp(tt, ogp, ost, o_pool):
                    ops = [o_pool.tile([P, TG], f32, name='op', tag='op')
                           for _ in range(2)]
                    for h in range(QH_PER_CORE):
                        for oi in range(2):
                            og = 2 * ogp + oi
                            nc.tensor.matmul(
                                ops[oi][:], yT[h][:, tt * P:(tt + 1) * P],
                                wp_sb[:, h, og * TG:(og + 1) * TG],
                                start=(h == 0),
                                stop=(h == QH_PER_CORE - 1),
                            )
                    for oi in range(2):
                        og = 2 * ogp + oi
                        if oi == 0:
                            nc.vector.tensor_copy(
                                ost[:, og * TG:(og + 1) * TG], ops[oi][:])
                        else:
                            nc.scalar.copy(
                                ost[:, og * TG:(og + 1) * TG], ops[oi][:])

                def proj_tts(tts, o_pool, ost_pool):
                    # software-pipelined pairs: og-pair blocks of two token
                    # tiles alternate so psum-bank reuse never waits a drain
                    for i in range(0, len(tts), 2):
                        pair = tts[i:i + 2]
                        osts = {tt: ost_pool.tile([P, C], bf16, name='ost')
                                for tt in pair}
                        for ogp in range(2):
                            for tt in pair:
                                _proj_ogp(tt, ogp, osts[tt], o_pool)
                        for tt in pair:
                            nc.sync.dma_start(
                                out=out_ap[tt * P:(tt + 1) * P, :],
                                in_=osts[tt][:])

                # ready-queue of projectable token tiles: all of batch 0 up
                # front; batch-1 tiles join one group after their y is final
                ready = list(range(NJB))
                pending = []
                for g in range(NGB):
                    for h in range(QH_PER_CORE):
                        attn_group(1, g, (h,), y1_ps, d1_ps, ilv_den=False)
                        ready.extend(pending)
                        pending = []
                        if h == 1:
                            pending = [NJB + 4 * g + i for i in range(4)]
                        take = min(4, len(ready))
                        proj_tts([ready.pop(0) for _ in range(take)],
                                 o2_ps, ost2p)
                proj_tts(ready, o2_ps, ost2p)
                leftover = pending

        # ------------- stage 3: output projection (tail tiles) -------------
        with ExitStack() as s4:
            o_ps = s4.enter_context(tc.tile_pool(name="ops", bufs=8, space="PSUM"))
            ostgp = s4.enter_context(tc.tile_pool(name="ostg", bufs=3))
            for tt in leftover:
                ost = ostgp.tile([P, C], bf16, name='ost')
                ops = [o_ps.tile([P, TG], f32, name=f'op{_og}', tag='op') for _og in range(C // TG)]
                for h in range(QH_PER_CORE):
                    for og in range(C // TG):
                        nc.tensor.matmul(
                            ops[og][:], yT[h][:, tt * P:(tt + 1) * P],
                            wp_sb[:, h, og * TG:(og + 1) * TG],
                            start=(h == 0), stop=(h == QH_PER_CORE - 1),
                        )
                for og in range(C // TG):
                    if og % 2 == 0:
                        nc.vector.tensor_copy(
                            ost[:, og * TG:(og + 1) * TG], ops[og][:])
                    else:
                        nc.scalar.copy(
                            ost[:, og * TG:(og + 1) * TG], ops[og][:])
                nc.sync.dma_start(out=out_ap[tt * P:(tt + 1) * P, :],
                                  in_=ost[:])


def build_nc():
    """Build and compile the (single, shared across cores) Bass program."""
    if "nc" in _CACHE:
        return _CACHE["nc"]
    import concourse.mybir as mybir
    import concourse.tile as tile
    from concourse import bacc

    f32 = mybir.dt.float32  # noqa: F841
    bf16 = mybir.dt.bfloat16

    nc = bacc.Bacc("TRN2", target_bir_lowering=False, debug=False)
    shapes = {
        "x_sw": ((P, KT, NT, TG), bf16),
        "wq_sw": ((P, KT, TG), bf16),
        "wp_sw": ((P, QH_PER_CORE, C), bf16),
        "cs_sw": ((P, 2, T), bf16),
        "mask_sw": ((P, P), bf16),
        "eye_sw": ((P, P), bf16),
    }
    t_in = {
        name: nc.dram_tensor(name, shape, dt, kind="ExternalInput").ap()
        for name, (shape, dt) in shapes.items()
    }
    out_ap = nc.dram_tensor("out", (TOK, C), bf16, kind="ExternalOutput").ap()

    with tile.TileContext(nc) as tc:
        _emit(tc, out_ap, t_in)
    nc.compile()
    _CACHE["nc"] = nc
    return nc


# --------------------------------------------------------------------------
# host-side data preparation
# --------------------------------------------------------------------------

def _swizzle_ktiles(a2d):
    """[R*128, F] -> [128, R, F] picking partition-within-tile as leading."""
    r128, f = a2d.shape
    r = r128 // P
    return np.ascontiguousarray(a2d.reshape(r, P, f).transpose(1, 0, 2))


def host_prep(x, w_attn, w_proj, cos, sin):
    x = np.asarray(x, np.float32)
    w_attn = np.asarray(w_attn, np.float32)
    w_proj = np.asarray(w_proj, np.float32)
    cos = np.asarray(cos, np.float32).reshape(T, HD // 2)
    sin = np.asarray(sin, np.float32).reshape(T, HD // 2)

    # x: (B,T,C) -> xT (C, TOK) -> [128, n, k, t]
    xT = x.reshape(TOK, C).T                        # (C, TOK)
    x_sw = (
        xT.reshape(KT, P, NT, TG).transpose(1, 0, 2, 3)  # (P, k, n, t)
    )
    x_sw = np.ascontiguousarray(x_sw).astype(BF16)

    # cos/sin duplicated across both 64-partition halves: [128, 2, T]
    c2 = np.concatenate([cos.T, cos.T], axis=0)     # (128, T)
    s2 = np.concatenate([sin.T, -sin.T], axis=0)    # sign-folded for rope add
    cs_sw = np.stack([c2, s2], axis=1).astype(BF16)  # (128, 2, T)

    # within-block causal mask: keep col >= row
    col = np.arange(P)[None, :]
    row = np.arange(P)[:, None]
    mask_sw = (col >= row).astype(BF16)              # (128, 128)

    eye_sw = np.eye(P, dtype=np.float32).astype(BF16)

    in_maps = []
    for c in range(N_CORES):
        qrows = w_attn[QH_PER_CORE * HD * c: QH_PER_CORE * HD * (c + 1)]
        krows = w_attn[C + HD * c: C + HD * (c + 1)]
        vrows = w_attn[C + KV_DIM + HD * c: C + KV_DIM + HD * (c + 1)]
        w_sel = np.concatenate([qrows, krows, vrows], axis=0)   # (512, C)
        wq_sw = _swizzle_ktiles(w_sel.T).astype(BF16)           # (128, 16, 512)

        wp_sel = w_proj[:, QH_PER_CORE * HD * c: QH_PER_CORE * HD * (c + 1)]
        wp_sw = _swizzle_ktiles(np.ascontiguousarray(wp_sel.T)).astype(BF16)

        in_maps.append({
            "x_sw": x_sw,
            "wq_sw": np.ascontiguousarray(wq_sw.reshape(P, KT, TG)),
            "wp_sw": np.ascontiguousarray(wp_sw.reshape(P, QH_PER_CORE, C)),
            "cs_sw": cs_sw,
            "mask_sw": mask_sw,
            "eye_sw": eye_sw,
        })
    return in_maps


def run_on_hw(in_maps, trace=False, **kwargs):
    from concourse import bass_utils

    nc = build_nc()
    return bass_utils.run_bass_kernel_spmd(
        nc, in_maps, core_ids=list(range(N_CORES)), trace=trace, **kwargs
    )


def kernel(x, w_attn, w_proj, cos, sin):
    in_maps = host_prep(x, w_attn, w_proj, cos, sin)
    res = run_on_hw(in_maps)
    out = np.zeros((TOK, C), np.float64)
    for r in res.results:
        out += r["out"].astype(np.float64)
    return out.astype(np.float32).reshape(B, T, C)
